# revision 1
# baseline (speedup 1.0000x reference)
"""Trainium2 kernel for nn_BasicWHVILinear.

Math (reference):
    qf    = tril(Q) + tril(Q)^T - diag(diag(Q))        (symmetric, 2048x2048)
    Sigma = qf @ qf^T
    L     = cholesky(Sigma)
    g     = q_mu + L @ eps
    u     = H^T @ (s1 * g)                              (H = scaled Hadamard)
    W     = s2[:,None] * H^T * u[None,:]
    out   = relu(x @ W^T),  x: (16384, 2048)

Sharding strategy (per spec hint): data-parallel on the batch axis — the
16384-row x is split into 8 shards of 2048 rows, one per NeuronCore; the
D-dim parameter pipeline (Sigma -> Cholesky -> g -> u -> W, ~7% of total
FLOPs, serial) is replicated preprocessing shared by every shard, and each
core runs the batched GEMM out_c = relu(x_c @ W^T) on device.

Device GEMM design notes (constraints of this walrus/bass toolchain):
  - PE Matmult and SP-issued HWDGE DMACopy instructions only support ONE
    semaphore wait each; walrus codegen hard-fails otherwise. Therefore:
      * every DMA lands in a write-once SBUF destination (no staging rings),
        so no DMA ever needs a prior-writer/reader wait on top of its own
        queue wait;
      * both GEMM operands live fully resident in SBUF in bf16 (8 MB + 8 MB),
        with a DVE self-copy "fence" over each DMA'd region so that every PE
        matmul depends only on the single DVE semaphore;
      * PSUM eviction (fused relu) also runs on DVE, keeping the
        start-of-accumulation matmuls single-wait as well.
  - bf16 operands at fp32 PSUM accumulation, with the output also emitted
    as bf16 and upcast to the fp32 contract on the host: 3.9e-3 relative
    error vs the fp64 oracle (validated off-line), ~5x inside the accuracy
    budget, and the writeback DMA traffic is halved.
  - x^T is pre-transposed and pre-cast on the host so both operands stream
    K-major; 16 MB in + 8 MB out per core sits well under the PE time
    (~224 us).
"""

import os
import numpy as np

D = 2048
BATCH = 16384
N_CORES = 8
ROWS = BATCH // N_CORES  # rows of x per core

P = 128
KT = D // P          # 16 contraction tiles
NQ = 512             # psum free dim (one bank)
NT = D // NQ         # 4 n-chunks
MT = ROWS // P       # 16 output row tiles per core
MCH = 512            # m-chunk for x loads
MCT = ROWS // MCH    # 4 m-chunks

TRACE = bool(int(os.environ.get("WHVI_KERNEL_TRACE", "0")))
LAST_EXEC_TIME_NS = None
LAST_RESULT = None

_PROGRAM = None


def _build_H():
    H = np.array([[1.0, 1.0], [1.0, -1.0]], dtype=np.float32)
    while H.shape[0] < D:
        H = np.block([[H, H], [H, -H]])
    return H * np.float32(D ** -0.5)


def _host_wt(s1, s2, q_mu, q_factor_lower, eps):
    """Replicated parameter pipeline -> W^T (K x N layout for the GEMM)."""
    ql = np.asarray(q_factor_lower, np.float32)
    qf = ql + ql.T - np.diag(np.diag(ql))
    Sigma = qf @ qf.T
    L = np.linalg.cholesky(Sigma)
    g = np.asarray(q_mu, np.float32) + L @ np.asarray(eps, np.float32)
    H = _build_H()
    u = H.T @ (np.asarray(s1, np.float32) * g)
    # W[i, j] = s2[i] * H[j, i] * u[j]  =>  W^T[j, i] = u[j] * H[j, i] * s2[i]
    WT = u[:, None] * H * np.asarray(s2, np.float32)[None, :]
    return np.ascontiguousarray(WT, dtype=np.float32)


def _build_program():
    from contextlib import ExitStack

    import concourse.bacc as bacc
    import concourse.mybir as mybir
    import concourse.tile as tile

    f32 = mybir.dt.float32
    bf16 = mybir.dt.bfloat16

    # Bacc (not raw Bass): its finalize() runs generate_event_semaphores /
    # fuse_nops, which split multi-semaphore waits into EventSemaphore
    # instructions — this walrus only accepts ONE wait per instruction.
    nc = bacc.Bacc()
    xT = nc.declare_dram_parameter("xT", [D, ROWS], bf16, isOutput=False)
    wt = nc.declare_dram_parameter("wt", [D, D], bf16, isOutput=False)
    out = nc.declare_dram_parameter("out", [ROWS, D], bf16, isOutput=True)

    with tile.TileContext(nc) as tc:
        with ExitStack() as ctx:
            big_pool = ctx.enter_context(tc.tile_pool(name="big", bufs=1))
            out_pool = ctx.enter_context(tc.tile_pool(name="outs", bufs=2))
            psum_pool = ctx.enter_context(
                tc.tile_pool(name="psum", bufs=2, space="PSUM")
            )

            # Write-once resident operands.
            wtf = big_pool.tile([P, KT, NT, NQ], bf16)   # 8 MB
            xtf = big_pool.tile([P, KT, ROWS], bf16)     # 8 MB

            wt_v = wt[:].rearrange("(kt p) (n nq) -> p kt n nq", p=P, nq=NQ)
            xT_v = xT[:].rearrange("(kt p) m -> p kt m", p=P)

            # Only 8 physical HWDGE queues exist and queue assignment is
            # global round-robin; a 9th DMA wraps onto a used queue and picks
            # up a ring wait that walrus cannot encode next to a real dep.
            # Budget: 2 wt DMAs + 2 x chunks + 4 out DMAs = exactly 8.
            # The first compute slice (wt n=0, x m-cols 0:512) loads via small
            # DMAs so m=0 matmuls start ~10us in instead of ~40us.
            # First-slice loads on two different engines so the inline
            # DIRECT2D transfers overlap instead of serializing on SP.
            nc.sync.dma_start(wtf[:, :, 0, :], wt_v[:, :, 0, :])
            nc.sync.dma_start(xtf[:, :, 0:512], xT_v[:, :, 0:512])
            nc.sync.dma_start(wtf[:, :, 1:, :], wt_v[:, :, 1:, :])
            nc.sync.dma_start(xtf[:, :, 512:], xT_v[:, :, 512:])
            # DVE fences, first-compute slices first.
            nc.vector.tensor_copy(wtf[:, :, 0, :], wtf[:, :, 0, :])
            nc.vector.tensor_copy(xtf[:, :, 0:512], xtf[:, :, 0:512])
            for n in range(1, NT):
                nc.vector.tensor_copy(wtf[:, :, n, :], wtf[:, :, n, :])
            nc.vector.tensor_copy(xtf[:, :, 512:], xtf[:, :, 512:])

            # out is written back in 4 big DMAs (4 m-tiles each) on the
            # scalar engine — with the 4 input DMAs that is exactly the 8
            # physical HWDGE queues, so no DMA needs a queue-ring wait on
            # top of its DVE dep.
            CHUNKS = [4, 4, 4, 2, 2]
            mbase = 0
            for mb in CHUNKS:
                ot = out_pool.tile([P, 4, D], bf16, tag="ot", name="ot")
                for mloc in range(mb):
                    m = mbase + mloc
                    msl = slice(m * P, (m + 1) * P)
                    psums = [
                        psum_pool.tile([P, NQ], f32, tag=f"ps{n}", name=f"ps{n}")
                        for n in range(NT)
                    ]
                    for k in range(KT):
                        for n in range(NT):
                            nc.tensor.matmul(
                                psums[n][:],
                                xtf[:, k, msl],
                                wtf[:, k, n, :],
                                start=(k == 0),
                                stop=(k == KT - 1),
                            )
                    for n in range(NT):
                        nc.vector.tensor_scalar_max(
                            ot[:, mloc, n * NQ : (n + 1) * NQ], psums[n][:], 0.0
                        )
                out_rows = out[mbase * P : (mbase + mb) * P, :]
                nc.scalar.dma_start(
                    out_rows.rearrange("(mt p) n -> p mt n", p=P), ot[:, :mb, :]
                )
                mbase += mb
    nc.finalize()
    return nc


def kernel(x, s1, s2, q_mu, q_factor_lower, eps):
    global _PROGRAM, LAST_EXEC_TIME_NS, LAST_RESULT
    import ml_dtypes
    from concourse.bass_utils import run_bass_kernel_spmd

    bf16 = ml_dtypes.bfloat16
    x = np.asarray(x, np.float32)
    WT = _host_wt(s1, s2, q_mu, q_factor_lower, eps).astype(bf16)

    if _PROGRAM is None:
        _PROGRAM = _build_program()

    core_ids = list(range(N_CORES))
    in_maps = [
        {
            "xT": np.ascontiguousarray(x[c * ROWS : (c + 1) * ROWS].T.astype(bf16)),
            "wt": WT,
        }
        for c in core_ids
    ]
    res = run_bass_kernel_spmd(_PROGRAM, in_maps, core_ids, trace=TRACE)
    LAST_RESULT = res
    LAST_EXEC_TIME_NS = res.exec_time_ns
    out = np.concatenate(
        [np.asarray(res.results[c]["out"]) for c in core_ids], axis=0
    )
    # device emits bf16 (halves the writeback DMA); upcast to the fp32 contract
    return np.ascontiguousarray(out.astype(np.float32))



# revision 7
# speedup vs baseline: 2.9220x; 2.9220x over previous
"""Trainium2 kernel for nn_BasicWHVILinear.

Math (reference):
    qf    = tril(Q) + tril(Q)^T - diag(diag(Q))        (symmetric, 2048x2048)
    Sigma = qf @ qf^T ; L = cholesky(Sigma) ; g = q_mu + L @ eps
    u     = H^T @ (s1 * g)                              (H = scaled Hadamard)
    W     = s2[:,None] * H^T * u[None,:]
    out   = relu(x @ W^T),  x: (16384, 2048)

Key structure: W^T = u[:,None] * H * s2  =>  out^T = relu(s2 ⊙ (H @ (u ⊙ x^T)))
with H = 2048^-1/2 * Hadamard(2048). The dense GEMM is therefore a scaled
Walsh-Hadamard transform. Kronecker-factoring H2048 = H16 (x) H128 turns the
2*2048^3-FLOP GEMM into two thin matmul stages (16x fewer MACs), moving the
kernel from PE-bound (~218 us roofline) to DMA-bound (~46 us).

Sharding: data-parallel over the batch axis, 2048 rows of x per core; the
D-dim parameter pipeline (Sigma -> cholesky -> g -> u, plus the small
Hadamard-factor stationaries) is replicated host-side per the spec hint.

Device pipeline per core (m split into 4 chunks of 512 columns; each chunk is
64 "k-units" of 8 m-columns; partition index p = j1*8 + m8, j = j1*128 + j2):
  1. u-fence (DVE + GpSimd split): Xs = X_pre ⊙ U_b in place — doubles as the
     DMA fence so downstream PE waits collapse to one semaphore.
  2. mm1 x64: matmul(lhsT=Xs_unit[(j1,m8), j2] as the *stationary*, rhs=G)
     with G = kron(Had16, I8)/4. Swapping the operand roles makes the output
     land directly in transposed layout psum[(j2), (i1,m8)] — this replaces
     a separate PE-transpose stage AND its eviction pass.
  3. E_T (Act): evict psum -> bf16 SBUF T'[j2, k, (i1,m8)].
  4. mm2 x16: matmul(lhsT=W2_i1[j2,i2] = Had128*2^-3.5*s2-slice, rhs=T'
     windows of fixed i1) -> psum_Y[i2, (k,m8)]. s2 is folded into the 16
     stationaries so the final eviction is a plain relu.
  5. E3 (DVE): tensor_scalar_max(psum, 0) -> bf16 y[i2, (i1, m)].
  6. DMA out to outT[(i1*128+i2), m] (1 KiB contiguous runs).

DMA budget is exactly the 8 physical HWDGE queues: 1 combined consts+chunk0
load, 3 more x-chunk loads (3-deep ping-pong so no load waits on compute
further than one chunk back), 4 y stores.
"""

import os
import numpy as np

D = 2048
BATCH = 16384
N_CORES = 8
ROWS = BATCH // N_CORES   # 2048 rows of x per core

P = 128
J1 = 16                   # Had16 factor
M8 = 8                    # m-columns packed per partition group
KTOT = ROWS // M8         # 256 k-units per core
NCHUNK = 4
KC = KTOT // NCHUNK       # 64 k-units per chunk
MC = KC * M8              # 512 m-columns per chunk

CST = 128 + 128 + 16 * 128   # G | U_b | W2[16]  (bf16 cols)
XFREE = KTOT * P             # x free size per partition (32768)

# DVE/GpSimd split of the 64-unit u-fence (DVE is ~3.8x faster per element)
KC_DVE = 50
TRACE = bool(int(os.environ.get("WHVI_KERNEL_TRACE", "0")))
LAST_EXEC_TIME_NS = None
LAST_RESULT = None
_PROGRAM = None


def _had(n):
    M = np.array([[1.0]], dtype=np.float64)
    while M.shape[0] < n:
        M = np.block([[M, M], [M, -M]])
    return M


def _host_params(s1, s2, q_mu, q_factor_lower, eps):
    """Replicated parameter pipeline -> (u, s2) then the device stationaries."""
    ql = np.asarray(q_factor_lower, np.float32)
    qf = ql + ql.T - np.diag(np.diag(ql))
    Sigma = qf @ qf.T
    L = np.linalg.cholesky(Sigma)
    g = np.asarray(q_mu, np.float32) + L @ np.asarray(eps, np.float32)
    H = (_had(D) * (D ** -0.5)).astype(np.float32)
    u = H.T @ (np.asarray(s1, np.float32) * g)
    return u.astype(np.float32), np.asarray(s2, np.float32)


def _build_consts(u, s2, bf16):
    """[128, CST] bf16: G | U_b | W2[16] (column-blocks of 128)."""
    H16 = _had(J1)
    H128 = _had(P)
    cst = np.zeros((P, CST), dtype=np.float32)
    # G[(j1,m8), (i1,m8')] = Had16[j1,i1]/4 * delta(m8,m8')
    cst[:, 0:128] = np.kron(H16, np.eye(M8)) * 0.25
    # U_b[(j1,m8), j2] = u[j1*128 + j2]
    cst[:, 128:256] = np.repeat(u.reshape(J1, P), M8, axis=0)
    # W2_i1[j2, i2] = Had128[j2,i2] * 128^-0.5 * s2[i1*128+i2]
    scale = P ** -0.5
    for i1 in range(J1):
        cst[:, 256 + i1 * P : 256 + (i1 + 1) * P] = (
            H128 * scale * s2[i1 * P : (i1 + 1) * P][None, :]
        )
    return cst.astype(bf16)


def _host_xpre(xc, bf16):
    """x core block (2048, 2048) -> [p=(j1,m8), k, j2] bf16, flattened."""
    # row m = k*8+m8, col j = j1*128+j2
    xp = xc.reshape(KTOT, M8, J1, P).transpose(2, 1, 0, 3).reshape(P, KTOT * P)
    return np.ascontiguousarray(xp.astype(bf16))


def _build_program():
    from contextlib import ExitStack

    import concourse.bacc as bacc
    import concourse.mybir as mybir
    import concourse.tile as tile

    f32 = mybir.dt.float32
    bf16 = mybir.dt.bfloat16
    Relu = mybir.ActivationFunctionType.Relu
    Copy = mybir.ActivationFunctionType.Copy

    nc = bacc.Bacc()
    # host packs: [cst (CST cols) | x chunk0 | chunk1 | chunk2 | chunk3]
    xp = nc.declare_dram_parameter("xp", [P, CST + XFREE], bf16, isOutput=False)
    out = nc.declare_dram_parameter("out", [D, ROWS], bf16, isOutput=True)

    XS_SLOT = KC * P  # 8192 free elems per x slot

    with tile.TileContext(nc) as tc:
        with ExitStack() as ctx:
            sb = ctx.enter_context(tc.tile_pool(name="sb", bufs=1))
            psum_pool = ctx.enter_context(
                tc.tile_pool(name="psum", bufs=1, space="PSUM")
            )

            # one tensor so consts + x-slot0 can share a single DMA
            big = sb.tile([P, CST + 3 * XS_SLOT], bf16)     # 55.1 KB/part
            tp = sb.tile([P, 2, KC, P], bf16)               # T' ping-pong
            yb = sb.tile([P, 2, J1, MC], bf16)              # y ping-pong

            cst = big[:, 0:CST]
            G = cst[:, 0:128]
            Ub = cst[:, 128:256]

            def w2(i1):
                return cst[:, 256 + i1 * P : 256 + (i1 + 1) * P]

            def xslot(s):
                base = CST + s * XS_SLOT
                return big[:, base : base + XS_SLOT].rearrange(
                    "p (k j2) -> p k j2", j2=P
                )

            # psum: 4 banks for mm1 output ring (4 units each), 2x2 for mm2
            psT = [psum_pool.tile([P, 512], f32, name=f"pt{b}") for b in range(4)]
            psY = [psum_pool.tile([P, 1024], f32, name=f"py{b}") for b in range(2)]

            # --- DMAs: exactly 8 HWDGE queues ---
            # inputs on SP, outputs on Act (baseline-proven pattern).
            # Chunk 3 reuses slot 0, so its load is issued inside the loop
            # AFTER chunk 0's compute — Tile tracks deps in program order, so
            # issuing it here would make chunk 0 read chunk 3's data.
            nc.sync.dma_start(big[:, 0 : CST + XS_SLOT], xp[:, 0 : CST + XS_SLOT])
            for c in (1, 2):
                s = c % 3
                src = xp[:, CST + c * XS_SLOT : CST + (c + 1) * XS_SLOT]
                nc.sync.dma_start(big[:, CST + s * XS_SLOT : CST + (s + 1) * XS_SLOT], src)

            out_v = out[:].rearrange("(i1 p) m -> p i1 m", p=P)

            for c in range(NCHUNK):
                s = c % 3
                pp = c % 2
                xs = xslot(s)
                if c == 1:
                    # chunk-3 load into slot 0, after chunk 0's readers
                    nc.sync.dma_start(
                        big[:, CST : CST + XS_SLOT],
                        xp[:, CST + 3 * XS_SLOT : CST + 4 * XS_SLOT],
                    )
                # u-fence: in-place multiply by U_b (broadcast over k); also
                # the single-semaphore DMA fence for this chunk's x data.
                ub_d = Ub.unsqueeze(1).broadcast_to([P, KC_DVE, P])
                nc.vector.tensor_mul(
                    xs[:, 0:KC_DVE, :], xs[:, 0:KC_DVE, :], ub_d
                )
                ub_g = Ub.unsqueeze(1).broadcast_to([P, KC - KC_DVE, P])
                nc.gpsimd.tensor_mul(
                    xs[:, KC_DVE:KC, :], xs[:, KC_DVE:KC, :], ub_g
                )

                # mm1: 64 data-stationary matmuls -> transposed layout
                for kk in range(KC):
                    bank = (kk // 4) % 4
                    q = kk % 4
                    nc.tensor.matmul(
                        psT[bank][:, q * P : (q + 1) * P],
                        xs[:, kk, :],       # stationary [(j1,m8), j2]
                        G,                  # moving     [(j1,m8), (i1,m8)]
                        start=True,
                        stop=True,
                    )
                    # E_T: evict 2 banks (8 units) per pair of Act instrs
                    if kk % 8 == 7:
                        g8 = kk // 8
                        b0 = (g8 * 2) % 4
                        nc.scalar.activation(
                            tp[:, pp, g8 * 8 : g8 * 8 + 4, :].rearrange(
                                "p a b -> p (a b)"
                            ),
                            psT[b0][:, :],
                            Copy,
                        )
                        nc.scalar.activation(
                            tp[:, pp, g8 * 8 + 4 : g8 * 8 + 8, :].rearrange(
                                "p a b -> p (a b)"
                            ),
                            psT[(b0 + 1) % 4][:, :],
                            Copy,
                        )

                # mm2: 16 matmuls, one per i1 output block (s2 folded in)
                for i1 in range(J1):
                    half = i1 % 2
                    nc.tensor.matmul(
                        psY[(i1 // 2) % 2][:, half * 512 : (half + 1) * 512],
                        w2(i1),                        # stationary [j2, i2]
                        tp[:, pp, :, i1 * M8 : (i1 + 1) * M8],  # moving [j2,(k,m8)]
                        start=True,
                        stop=True,
                    )
                    # E3: relu-evict a psY tile (2 i1 blocks) per DVE instr
                    if i1 % 2 == 1:
                        nc.vector.tensor_scalar_max(
                            yb[:, pp, i1 - 1 : i1 + 1, :].rearrange(
                                "p a m -> p (a m)"
                            ),
                            psY[(i1 // 2) % 2][:, :],
                            0.0,
                        )

                nc.scalar.dma_start(
                    out_v[:, :, c * MC : (c + 1) * MC], yb[:, pp, :, :]
                )
    nc.finalize()
    return nc


def kernel(x, s1, s2, q_mu, q_factor_lower, eps):
    global _PROGRAM, LAST_EXEC_TIME_NS, LAST_RESULT
    import ml_dtypes
    from concourse.bass_utils import run_bass_kernel_spmd

    bf16 = ml_dtypes.bfloat16
    x = np.asarray(x, np.float32)
    u, s2f = _host_params(s1, s2, q_mu, q_factor_lower, eps)
    cst = _build_consts(u, s2f, bf16)

    if _PROGRAM is None:
        _PROGRAM = _build_program()

    core_ids = list(range(N_CORES))
    in_maps = []
    for c in core_ids:
        xpre = _host_xpre(x[c * ROWS : (c + 1) * ROWS], bf16)
        in_maps.append({"xp": np.ascontiguousarray(np.concatenate([cst, xpre], axis=1))})
    res = run_bass_kernel_spmd(_PROGRAM, in_maps, core_ids, trace=TRACE)
    LAST_RESULT = res
    LAST_EXEC_TIME_NS = res.exec_time_ns
    # device emits outT [i, m] bf16 per core; transpose + upcast on host
    outs = [
        np.asarray(res.results[c]["out"]).astype(np.float32).T for c in core_ids
    ]
    return np.ascontiguousarray(np.concatenate(outs, axis=0))
